# revision 12
# baseline (speedup 1.0000x reference)
"""Multi-headed attention (B=2, S=2048, D=1024, H=16) on 8 TRN2 NeuronCores.

Sharding: tensor-parallel over heads for the attention body (2 heads/core,
both batches on every core), then AllToAll reshards to (batch, seq-quarter)
for the output projection. Per core:

  1. K/V/Q projections (bf16 matmuls, fp32 psum):
       qhT/khT [128e, 2048s] (e on partitions), vh [2048t, 65e'] per head
       with a ones column appended (row 64 of the AV output = rowsum).
  2. logits^T = khT-tiles.T @ qhT  (K=64, two heads row-packed, row-tiled
     concurrent on the PE).
  3. P = exp(0.125 * logits^T) on ScalarE (PSUM -> SBUF bf16, FD=1024).
     The AV consumption of P runs one t-tile BEHIND the logits production
     so the ScalarE exp stream never waits on the PE's in-order queue.
  4. A_h[0:65] += [vh_h | 1].T @ P_h per head, accumulated over 16 t-tiles;
     row 64 = softmax denominator.
  5. A copied to SBUF (frees psum for the next chunk), rec =
     approx-reciprocal of row 64, partition-broadcast (GpSimd),
     heads^T *= rec -> hN bf16.
  6. Two AllToAlls (one per batch, zero-padded blocks for the other batch's
     ranks). The receiver sums the two output projections (one is zeros).
  7. out = gelu_sigmoid(heads_full^T-tiles.T @ Wo + bo) -> [512, 1024] f32
     = (batch r//4, seq-quarter r%4) slab of the full output. The batch-0
     half of the contraction is dripped into batch-1's attention loop;
     small dummy matmuls pad the second AllToAll's wait so the batch-1
     half starts at full clock.

All other-batch projection work is dripped into the attention loops in
small steps (on the spare PSUM "A" slot, never the logits slots) so the
Tensor engine stays dense and the HAM clock stays at full rate.
"""

import numpy as np
import ml_dtypes

import concourse.bass as bass
import concourse.mybir as mybir
import concourse.tile as tile
from concourse import bacc
from concourse.bass_utils import run_bass_kernel_spmd

F = mybir.ActivationFunctionType
BF16 = mybir.dt.bfloat16
F32 = mybir.dt.float32
BF = ml_dtypes.bfloat16

B, S, D, H = 2, 2048, 1024, 16
HD = D // H
NCORES = 8
SQ = S // 4
KT = D // 128
TT = S // 128
SC = S // 512

_CACHE = {}


def _build():
    nc = bacc.Bacc("TRN2", target_bir_lowering=False, debug=False,
                   num_devices=NCORES)
    xq = [nc.dram_tensor(f"xq{b}", [D, S], BF16, kind="ExternalInput") for b in range(B)]
    xk = [nc.dram_tensor(f"xk{b}", [D, S], BF16, kind="ExternalInput") for b in range(B)]
    xv = [nc.dram_tensor(f"xv{b}", [D, S], BF16, kind="ExternalInput") for b in range(B)]
    wq_d = nc.dram_tensor("wq", [D, 128], BF16, kind="ExternalInput")
    wk_d = nc.dram_tensor("wk", [D, 128], BF16, kind="ExternalInput")
    wv_d = nc.dram_tensor("wv", [D, 128], BF16, kind="ExternalInput")
    bq_d = nc.dram_tensor("bq", [128, 1], F32, kind="ExternalInput")
    bk_d = nc.dram_tensor("bk", [128, 1], F32, kind="ExternalInput")
    bv_d = nc.dram_tensor("bv", [1, 128], BF16, kind="ExternalInput")
    wo_d = nc.dram_tensor("wo", [D, D], BF16, kind="ExternalInput")
    bo_d = nc.dram_tensor("bo", [1, D], BF16, kind="ExternalInput")
    onr_d = nc.dram_tensor("onr", [1, 128], BF16, kind="ExternalInput")
    out_d = nc.dram_tensor("out", [SQ, D], F32, kind="ExternalOutput")

    xqr = [xq[b][:, :].rearrange("(kt p) s -> kt p s", p=128) for b in range(B)]
    xkr = [xk[b][:, :].rearrange("(kt p) s -> kt p s", p=128) for b in range(B)]
    xvr = [xv[b][:, :].rearrange("(kt p) s -> kt p s", p=128) for b in range(B)]

    with tile.TileContext(nc) as tc:
        with tc.tile_pool(name="cst", bufs=1) as cst, \
             tc.tile_pool(name="act", bufs=1) as acp, \
             tc.tile_pool(name="str", bufs=4) as stp, \
             tc.tile_pool(name="s2", bufs=3) as s2p, \
             tc.tile_pool(name="ps", bufs=2, space="PSUM") as ps, \
             tc.tile_pool(name="dram", bufs=1, space="DRAM") as dp:

            # k weights + k input loads first so the first projection can
            # start as early as possible
            wqt = cst.tile([128, KT, 128], BF16, tag="wqt")
            wkt = cst.tile([128, KT, 128], BF16, tag="wkt")
            wvt = cst.tile([128, KT, 128], BF16, tag="wvt")
            nc.sync.dma_start(wkt[:, :, :], wk_d[:, :].rearrange("(kt p) e -> p kt e", p=128))

            # ---------- emission helpers ----------
            def kproj_full(b):
                """K projection, full-row staging ([128,2048] chunks: 4KB
                contiguous per partition row -> efficient DMA)."""
                state = {}

                def load():
                    state["xc"] = []
                    for kt in range(KT):
                        xc = stp.tile([128, 2048], BF16, tag="xkc", bufs=8,
                                      name=f"xkc{b}{kt}")
                        nc.sync.dma_start(xc[:, :], xkr[b][kt, :, :])
                        state["xc"].append(xc)
                yield load

                for piece in range(4):
                    def palloc(piece=piece):
                        state["P"] = ps.tile([128, 512], F32, tag="A",
                                             name=f"xkp{b}{piece}")
                        for kt in range(0, 4):
                            nc.tensor.matmul(state["P"][:, :], wkt[:, kt, :],
                                             state["xc"][kt][:, piece * 512:(piece + 1) * 512],
                                             start=(kt == 0), stop=False)
                    yield palloc

                    def pfin(piece=piece):
                        P = state["P"]
                        for kt in range(4, KT):
                            nc.tensor.matmul(P[:, :], wkt[:, kt, :],
                                             state["xc"][kt][:, piece * 512:(piece + 1) * 512],
                                             start=False, stop=(kt == KT - 1))
                        off = piece * 512
                        nc.vector.tensor_scalar_add(khT[b][:, off:off + 512],
                                                    P[:, :], bkt[:, 0:1])
                    yield pfin

            def qproj_steps(b, sp):
                """Q projection for one 1024-wide s-half, as drip steps."""
                state = {}

                def load():
                    state["xc"] = []
                    for kt in range(KT):
                        xc = stp.tile([128, 1024], BF16, tag="xqc", bufs=10,
                                      name=f"xqc{b}{sp}{kt}")
                        nc.sync.dma_start(xc[:, :],
                                          xqr[b][kt, :, sp * 1024:(sp + 1) * 1024])
                        state["xc"].append(xc)
                yield load

                for half in range(2):
                    def palloc(half=half):
                        state["P"] = ps.tile([128, 512], F32, tag="A",
                                             name=f"xqp{b}{sp}{half}")
                        for kt in range(0, 4):
                            nc.tensor.matmul(state["P"][:, :], wqt[:, kt, :],
                                             state["xc"][kt][:, half * 512:(half + 1) * 512],
                                             start=(kt == 0), stop=False)
                    yield palloc

                    def pfin(half=half):
                        P = state["P"]
                        for kt in range(4, KT):
                            nc.tensor.matmul(P[:, :], wqt[:, kt, :],
                                             state["xc"][kt][:, half * 512:(half + 1) * 512],
                                             start=False, stop=(kt == KT - 1))
                        off = sp * 1024 + half * 512
                        nc.vector.tensor_scalar_add(qhT[b][:, off:off + 512],
                                                    P[:, :], bqt[:, 0:1])
                    yield pfin

            def vload_steps(b):
                for kt in range(KT):
                    def mk(b=b, kt=kt):
                        nc.gpsimd.dma_start(vx[b][:, kt, :], xvr[b][kt, :, :])
                    yield mk

            def vproj_steps(b):
                for tt in range(TT):
                    state = {}

                    def s0(b=b, tt=tt, state=state):
                        state["Vp"] = ps.tile([128, 128], F32, tag="A",
                                              name=f"Vp{b}{tt}")
                        for kt in range(4):
                            nc.tensor.matmul(state["Vp"][:, :],
                                             vx[b][:, kt, tt * 128:(tt + 1) * 128],
                                             wvt[:, kt, :], start=(kt == 0), stop=False)
                    yield s0

                    def s1(b=b, tt=tt, state=state):
                        Vp = state["Vp"]
                        for kt in range(4, KT):
                            nc.tensor.matmul(Vp[:, :],
                                             vx[b][:, kt, tt * 128:(tt + 1) * 128],
                                             wvt[:, kt, :], start=False, stop=False)
                        nc.tensor.matmul(Vp[:, :], onr[0:1, :], bvt[0:1, :],
                                         start=False, stop=True)
                        nc.vector.tensor_copy(vht[b][0][:, tt, 0:64], Vp[:, 0:64])
                        nc.vector.tensor_copy(vht[b][1][:, tt, 0:64], Vp[:, 64:128])
                    yield s1

            def stage2(b, sc, filler=None, pace=1, fin_prev=None):
                """One (batch, seq-quarter) attention chunk. AV matmuls run
                one t-tile behind the logits so ScalarE's exp stream stays
                gapless; fillers are emitted before the AV pair so the PE
                works through them during the exp wait. The normalization of
                the PREVIOUS chunk (fin_prev) is emitted a few tiles into
                this chunk so its engine-queue entries never gate anything."""
                s0, s1 = sc * 512, (sc + 1) * 512
                A0 = ps.tile([128, 512], F32, tag="A0", bufs=1, name=f"A0{b}{sc}")
                A1 = ps.tile([128, 512], F32, tag="A1", bufs=1, name=f"A1{b}{sc}")
                Ps = [None] * TT

                def av(tt):
                    st, sp_ = (tt == 0), (tt == TT - 1)
                    nc.tensor.matmul(A0[0:65, :], vht[b][0][:, tt, :],
                                     Ps[tt][:, 0:512], start=st, stop=sp_)
                    nc.tensor.matmul(A1[0:65, :], vht[b][1][:, tt, :],
                                     Ps[tt][:, 512:1024], start=st, stop=sp_)

                for tt in range(TT):
                    t0, t1 = tt * 128, (tt + 1) * 128
                    L2 = ps.tile([128, 1024], F32, tag="L", name=f"L2{b}{sc}{tt}")
                    nc.tensor.matmul(L2[:, 0:512], khT[b][0:64, t0:t1],
                                     qhT[b][0:64, s0:s1], start=True, stop=True)
                    nc.tensor.matmul(L2[:, 512:1024], khT[b][64:128, t0:t1],
                                     qhT[b][64:128, s0:s1], start=True, stop=True)
                    P = s2p.tile([128, 1024], BF16, tag="P", bufs=4, name=f"P{b}{sc}{tt}")
                    nc.scalar.activation(P[:, :], L2[:, :], F.Exp, scale=0.125)
                    Ps[tt] = P
                    if filler is not None:
                        for _ in range(pace):
                            step = next(filler, None)
                            if step is not None:
                                step()
                    if tt > 0:
                        av(tt - 1)
                    if tt == 2 and fin_prev is not None:
                        fin_prev()
                av(TT - 1)
                # copy A to SBUF so the psum accumulators free quickly; the
                # rest of the normalization is deferred (returned closure)
                hA0 = s2p.tile([65, 512], F32, tag="hA0", bufs=3, name=f"hA0{b}{sc}")
                hA1 = s2p.tile([65, 512], F32, tag="hA1", bufs=3, name=f"hA1{b}{sc}")
                nc.vector.tensor_copy(hA0[:, :], A0[0:65, :])
                nc.vector.tensor_copy(hA1[:, :], A1[0:65, :])

                def fin():
                    rec = s2p.tile([1, 1024], F32, tag="rec", bufs=1, name=f"rec{b}{sc}")
                    nc.vector.reciprocal(rec[0:1, 0:512], hA0[64:65, :])
                    nc.vector.reciprocal(rec[0:1, 512:1024], hA1[64:65, :])
                    recB = s2p.tile([64, 1024], F32, tag="recB", bufs=1, name=f"recB{b}{sc}")
                    nc.gpsimd.partition_broadcast(recB[:, :], rec[0:1, :])
                    nc.vector.tensor_mul(hN[b][0:64, s0:s1], hA0[0:64, :], recB[:, 0:512])
                    nc.vector.tensor_mul(hN[b][64:128, s0:s1], hA1[0:64, :], recB[:, 512:1024])
                    nc.sync.dma_start(a2a_in[b][4 * b + sc, :, :], hN[b][:, s0:s1])
                return fin

            # ---------- setup + schedule ----------
            qhT = [acp.tile([128, S], BF16, tag=f"qhT{b}", name=f"qhT{b}") for b in range(B)]
            khT = [acp.tile([128, S], BF16, tag=f"khT{b}", name=f"khT{b}") for b in range(B)]
            vht = [[acp.tile([128, TT, 65], BF16, tag=f"vht{b}{h}", name=f"vht{b}{h}")
                    for h in range(2)] for b in range(B)]
            vx = [acp.tile([128, KT, S], BF16, tag="vx", name=f"vx{b}") for b in range(B)]
            hN = [acp.tile([128, S], BF16, tag=f"hN{b}", name=f"hN{b}") for b in range(B)]
            wot = cst.tile([128, KT, D], BF16, tag="wot")
            bqt = cst.tile([128, 1], F32, tag="bqt")
            bkt = cst.tile([128, 1], F32, tag="bkt")
            bvt = cst.tile([1, 128], BF16, tag="bvt")
            bot = cst.tile([1, D], BF16, tag="bot")
            onr = cst.tile([1, 128], BF16, tag="onr")
            zt = cst.tile([128, SQ], BF16, tag="zt")
            hf1 = acp.tile([128, NCORES, SQ], BF16, tag="hf1")
            hf2 = acp.tile([128, NCORES, SQ], BF16, tag="hf2")
            hfs = acp.tile([128, NCORES, SQ], BF16, tag="hfs")
            a2a_in = [dp.tile([NCORES, 128, SQ], BF16, tag=f"a2a_in{b}", name=f"a2a_in{b}")
                      for b in range(B)]
            a2a_out = [dp.tile([NCORES, 128, SQ], BF16, tag=f"a2a_out{b}", name=f"a2a_out{b}")
                       for b in range(B)]

            import itertools
            kgen0 = kproj_full(0)
            next(kgen0)()       # xk0 loads right behind wkt
            nc.sync.dma_start(bkt[:, :], bk_d[:, :])
            nc.sync.dma_start(wqt[:, :, :], wq_d[:, :].rearrange("(kt p) e -> p kt e", p=128))
            nc.sync.dma_start(bqt[:, :], bq_d[:, :])
            qgen00 = qproj_steps(0, 0)
            next(qgen00)()      # xq0 first-half loads
            nc.sync.dma_start(wvt[:, :, :], wv_d[:, :].rearrange("(kt p) e -> p kt e", p=128))
            for t, dtens in ((bvt, bv_d), (bot, bo_d), (onr, onr_d)):
                nc.sync.dma_start(t[:, :], dtens[:, :])
            nc.vector.memset(zt[:, :], 0.0)
            for b in range(B):
                for h in range(2):
                    nc.vector.memset(vht[b][h][:, :, 64:65], 1.0)
            for step in vload_steps(0):
                step()          # xv0 loads behind xq0-sp0
            for step in kgen0:
                step()          # k pieces
            for step in qgen00:
                step()          # q first-half pieces

            # zero-fill the other-batch blocks of a2a_in (must complete
            # before the collectives; emitted late so they don't compete
            # with the startup input loads)
            for b in range(B):
                for r in range(NCORES):
                    if r // 4 != b:
                        nc.gpsimd.dma_start(a2a_in[b][r, :, :], zt[:, :])

            # batch-0 attention; vproj(0) drips into sc0, everything
            # batch-1 (and q0 second half) into sc1-3. All of fillerA must
            # be emitted before stage2(1, 0), else the in-order PE queue
            # deadlocks.
            q01 = qproj_steps(0, 1)
            next(q01)()         # xq0 second-half loads (slots free mid-proj)
            fin00 = stage2(0, 0, itertools.chain(vproj_steps(0)), pace=2)
            kg1 = kproj_full(1)
            qg10 = qproj_steps(1, 0)
            # fillers ordered by when their input DMAs can land: q0-sp1 data
            # is resident, vx1/xk1/xq1 stream in behind it
            fillerA = itertools.chain(vload_steps(1),
                                      [next(kg1), next(qg10)],
                                      q01,
                                      vproj_steps(1),
                                      kg1,
                                      qg10)
            fin01 = stage2(0, 1, fillerA, pace=2, fin_prev=fin00)
            fin02 = stage2(0, 2, fillerA, pace=2, fin_prev=fin01)
            fin03 = stage2(0, 3, fillerA, pace=2, fin_prev=fin02)
            for step in fillerA:
                step()
            fillerB = itertools.chain(qproj_steps(1, 1))
            next(fillerB)()     # xq1 second-half loads fire early
            fin10 = stage2(1, 0, fin_prev=fin03)
            # AllToAll-0 emitted after fin03 so its gpsimd-queue entry never
            # sits ahead of fin03's partition_broadcast (deadlock otherwise)
            nc.gpsimd.collective_compute(
                "AllToAll", mybir.AluOpType.bypass,
                replica_groups=[list(range(NCORES))],
                ins=[a2a_in[0].opt()], outs=[a2a_out[0].opt()])
            nc.sync.dma_start(wot[:, :, :],
                              wo_d[:, :].rearrange("(kt p) n -> p kt n", p=128))
            fin11 = stage2(1, 1, fillerB, fin_prev=fin10)
            for step in fillerB:
                step()
            fin12 = stage2(1, 2, fin_prev=fin11)
            fin13 = stage2(1, 3, fin_prev=fin12)
            fin13()
            nc.gpsimd.collective_compute(
                "AllToAll", mybir.AluOpType.bypass,
                replica_groups=[list(range(NCORES))],
                ins=[a2a_in[1].opt()], outs=[a2a_out[1].opt()])

            # ---- tail: small dummy matmuls (N=128, fast to drain) keep the
            # PE warm through the second AllToAll's wait; then both halves'
            # AllToAll outputs are summed (one is zeros) and a single output
            # projection runs at full clock.
            W = ps.tile([128, 128], F32, tag="A0", bufs=1, name="Wwarm")
            for i in range(384):
                nc.tensor.matmul(W[:, :], wot[:, i % KT, 0:128],
                                 wot[:, i % KT, 128:256],
                                 start=(i == 0), stop=(i == 383))
            wsb = s2p.tile([128, 128], BF16, tag="wsb", bufs=1, name="wsb")
            nc.vector.tensor_copy(wsb[:, :], W[:, :])
            nc.sync.dma_start(a2a_in[0][0, :, 0:128], wsb[:, :])

            for p in range(NCORES):
                nc.sync.dma_start(hf1[:, p, :], a2a_out[0][p, :, :])
            for p in range(NCORES):
                nc.sync.dma_start(hf2[:, p, :], a2a_out[1][p, :, :])
            for st in range(4):
                nc.vector.tensor_add(hfs[:, :, st * 128:(st + 1) * 128],
                                     hf1[:, :, st * 128:(st + 1) * 128],
                                     hf2[:, :, st * 128:(st + 1) * 128])
                O = ps.tile([128, 1024], F32, tag="L", name=f"O2_{st}")
                for nn in range(2):
                    n0, n1 = nn * 512, (nn + 1) * 512
                    for kt in range(KT):
                        nc.tensor.matmul(O[:, n0:n1],
                                         hfs[:, kt, st * 128:(st + 1) * 128],
                                         wot[:, kt, n0:n1],
                                         start=(kt == 0), stop=False)
                    nc.tensor.matmul(O[:, n0:n1], onr[0:1, :], bot[0:1, n0:n1],
                                     start=False, stop=True)
                OG = s2p.tile([128, 1024], F32, tag="OG", bufs=1, name=f"OG{st}")
                nc.scalar.activation(OG[:, :], O[:, :], F.Gelu_apprx_sigmoid)
                nc.sync.dma_start(out_d[st * 128:(st + 1) * 128, :], OG[:, :])

    nc.compile()
    return nc


def _in_maps(q, k, v, Wq, bq, Wk, bk, Wv, bv, Wo, bo):
    xq = [np.ascontiguousarray(q[b].T).astype(BF) for b in range(B)]
    xk = [np.ascontiguousarray(k[b].T).astype(BF) for b in range(B)]
    xv = [np.ascontiguousarray(v[b].T).astype(BF) for b in range(B)]
    wo_bf = np.ascontiguousarray(Wo).astype(BF)
    bo_r = np.asarray(bo).reshape(1, D).astype(BF)
    onr = np.ones((1, 128), BF)
    in_maps = []
    for c in range(NCORES):
        hs = slice(2 * c, 2 * c + 2)
        im = {
            "wq": np.ascontiguousarray(Wq[hs].transpose(1, 0, 2).reshape(D, 128)).astype(BF),
            "wk": np.ascontiguousarray(Wk[hs].transpose(1, 0, 2).reshape(D, 128)).astype(BF),
            "wv": np.ascontiguousarray(Wv[hs].transpose(1, 0, 2).reshape(D, 128)).astype(BF),
            "bq": np.asarray(bq[hs]).reshape(128, 1).astype(np.float32),
            "bk": np.asarray(bk[hs]).reshape(128, 1).astype(np.float32),
            "bv": np.asarray(bv[hs]).reshape(1, 128).astype(BF),
            "wo": wo_bf, "bo": bo_r, "onr": onr,
        }
        for b in range(B):
            im[f"xq{b}"] = xq[b]
            im[f"xk{b}"] = xk[b]
            im[f"xv{b}"] = xv[b]
        in_maps.append(im)
    return in_maps


def kernel(q, k, v, mask, Wq, bq, Wk, bk, Wv, bv, Wo, bo):
    if "nc" not in _CACHE:
        _CACHE["nc"] = _build()
    nc = _CACHE["nc"]
    in_maps = _in_maps(q, k, v, Wq, bq, Wk, bk, Wv, bv, Wo, bo)
    res = run_bass_kernel_spmd(nc, in_maps, core_ids=list(range(NCORES)))
    out = np.empty((B, S, D), np.float32)
    for r in range(NCORES):
        bb, jj = r // 4, r % 4
        out[bb, jj * SQ:(jj + 1) * SQ, :] = res.results[r]["out"]
    return out


# revision 14
# speedup vs baseline: 1.0685x; 1.0685x over previous
"""Multi-headed attention (B=2, S=2048, D=1024, H=16) on 8 TRN2 NeuronCores.

Sharding: tensor-parallel over heads for the attention body (2 heads/core,
both batches on every core), then AllToAll reshards to (batch, seq-quarter)
for the output projection. Per core:

  1. K/V/Q projections (bf16 matmuls, fp32 psum):
       qhT/khT [128e, 2048s] (e on partitions), vh [2048t, 65e'] per head
       with a ones column appended (row 64 of the AV output = rowsum).
  2. logits^T = khT-tiles.T @ qhT  (K=64, two heads row-packed, row-tiled
     concurrent on the PE).
  3. P = exp(0.125 * logits^T) on ScalarE (PSUM -> SBUF bf16, FD=1024).
     The AV consumption of P runs one t-tile BEHIND the logits production
     so the ScalarE exp stream never waits on the PE's in-order queue.
  4. A_h[0:65] += [vh_h | 1].T @ P_h per head, accumulated over 16 t-tiles;
     row 64 = softmax denominator.
  5. A copied to SBUF (frees psum for the next chunk), rec =
     approx-reciprocal of row 64, partition-broadcast (GpSimd),
     heads^T *= rec -> hN bf16.
  6. Two AllToAlls (one per batch, zero-padded blocks for the other batch's
     ranks). The receiver sums the two output projections (one is zeros).
  7. out = gelu_sigmoid(heads_full^T-tiles.T @ Wo + bo) -> [512, 1024] f32
     = (batch r//4, seq-quarter r%4) slab of the full output. The batch-0
     half of the contraction is dripped into batch-1's attention loop;
     small dummy matmuls pad the second AllToAll's wait so the batch-1
     half starts at full clock.

All other-batch projection work is dripped into the attention loops in
small steps (on the spare PSUM "A" slot, never the logits slots) so the
Tensor engine stays dense and the HAM clock stays at full rate.
"""

import numpy as np
import ml_dtypes

import concourse.bass as bass
import concourse.mybir as mybir
import concourse.tile as tile
from concourse import bacc
from concourse.bass_utils import run_bass_kernel_spmd

F = mybir.ActivationFunctionType
BF16 = mybir.dt.bfloat16
F32 = mybir.dt.float32
BF = ml_dtypes.bfloat16

B, S, D, H = 2, 2048, 1024, 16
HD = D // H
NCORES = 8
SQ = S // 4
KT = D // 128
TT = S // 128
SC = S // 512

_CACHE = {}


def _build():
    nc = bacc.Bacc("TRN2", target_bir_lowering=False, debug=False,
                   num_devices=NCORES)
    xq = [nc.dram_tensor(f"xq{b}", [D, S], BF16, kind="ExternalInput") for b in range(B)]
    xk = [nc.dram_tensor(f"xk{b}", [D, S], BF16, kind="ExternalInput") for b in range(B)]
    xv = [nc.dram_tensor(f"xv{b}", [D, S], BF16, kind="ExternalInput") for b in range(B)]
    wq_d = nc.dram_tensor("wq", [D, 128], BF16, kind="ExternalInput")
    wk_d = nc.dram_tensor("wk", [D, 128], BF16, kind="ExternalInput")
    wv_d = nc.dram_tensor("wv", [D, 128], BF16, kind="ExternalInput")
    bq_d = nc.dram_tensor("bq", [128, 1], F32, kind="ExternalInput")
    bk_d = nc.dram_tensor("bk", [128, 1], F32, kind="ExternalInput")
    bv_d = nc.dram_tensor("bv", [1, 128], BF16, kind="ExternalInput")
    wo_d = nc.dram_tensor("wo", [D, D], BF16, kind="ExternalInput")
    bo_d = nc.dram_tensor("bo", [1, D], BF16, kind="ExternalInput")
    onr_d = nc.dram_tensor("onr", [1, 128], BF16, kind="ExternalInput")
    out_d = nc.dram_tensor("out", [SQ, D], F32, kind="ExternalOutput")

    xqr = [xq[b][:, :].rearrange("(kt p) s -> kt p s", p=128) for b in range(B)]
    xkr = [xk[b][:, :].rearrange("(kt p) s -> kt p s", p=128) for b in range(B)]
    xvr = [xv[b][:, :].rearrange("(kt p) s -> kt p s", p=128) for b in range(B)]

    with tile.TileContext(nc) as tc:
        with tc.tile_pool(name="cst", bufs=1) as cst, \
             tc.tile_pool(name="act", bufs=1) as acp, \
             tc.tile_pool(name="str", bufs=4) as stp, \
             tc.tile_pool(name="s2", bufs=3) as s2p, \
             tc.tile_pool(name="ps", bufs=2, space="PSUM") as ps, \
             tc.tile_pool(name="dram", bufs=1, space="DRAM") as dp:

            # k weights + k input loads first so the first projection can
            # start as early as possible
            wqt = cst.tile([128, KT, 128], BF16, tag="wqt")
            wkt = cst.tile([128, KT, 128], BF16, tag="wkt")
            wvt = cst.tile([128, KT, 128], BF16, tag="wvt")
            nc.sync.dma_start(wkt[:, :, :], wk_d[:, :].rearrange("(kt p) e -> p kt e", p=128))

            # ---------- emission helpers ----------
            def kproj_full(b):
                """K projection, full-row staging ([128,2048] chunks: 4KB
                contiguous per partition row -> efficient DMA)."""
                state = {}

                def load():
                    state["xc"] = []
                    for kt in range(KT):
                        xc = stp.tile([128, 2048], BF16, tag="xkc", bufs=8,
                                      name=f"xkc{b}{kt}")
                        nc.sync.dma_start(xc[:, :], xkr[b][kt, :, :])
                        state["xc"].append(xc)
                yield load

                for piece in range(4):
                    def palloc(piece=piece):
                        state["P"] = ps.tile([128, 512], F32, tag="A",
                                             name=f"xkp{b}{piece}")
                        for kt in range(0, 4):
                            nc.tensor.matmul(state["P"][:, :], wkt[:, kt, :],
                                             state["xc"][kt][:, piece * 512:(piece + 1) * 512],
                                             start=(kt == 0), stop=False)
                    yield palloc

                    def pfin(piece=piece):
                        P = state["P"]
                        for kt in range(4, KT):
                            nc.tensor.matmul(P[:, :], wkt[:, kt, :],
                                             state["xc"][kt][:, piece * 512:(piece + 1) * 512],
                                             start=False, stop=(kt == KT - 1))
                        off = piece * 512
                        nc.vector.tensor_scalar_add(khT[b][:, off:off + 512],
                                                    P[:, :], bkt[:, 0:1])
                    yield pfin

            def qproj_steps(b, sp):
                """Q projection for one 1024-wide s-half, as drip steps."""
                state = {}

                def load():
                    state["xc"] = []
                    for kt in range(KT):
                        xc = stp.tile([128, 1024], BF16, tag="xqc", bufs=10,
                                      name=f"xqc{b}{sp}{kt}")
                        nc.sync.dma_start(xc[:, :],
                                          xqr[b][kt, :, sp * 1024:(sp + 1) * 1024])
                        state["xc"].append(xc)
                yield load

                for half in range(2):
                    def palloc(half=half):
                        state["P"] = ps.tile([128, 512], F32, tag="A",
                                             name=f"xqp{b}{sp}{half}")
                        for kt in range(0, 4):
                            nc.tensor.matmul(state["P"][:, :], wqt[:, kt, :],
                                             state["xc"][kt][:, half * 512:(half + 1) * 512],
                                             start=(kt == 0), stop=False)
                    yield palloc

                    def pfin(half=half):
                        P = state["P"]
                        for kt in range(4, KT):
                            nc.tensor.matmul(P[:, :], wqt[:, kt, :],
                                             state["xc"][kt][:, half * 512:(half + 1) * 512],
                                             start=False, stop=(kt == KT - 1))
                        off = sp * 1024 + half * 512
                        nc.vector.tensor_scalar_add(qhT[b][:, off:off + 512],
                                                    P[:, :], bqt[:, 0:1])
                    yield pfin

            def vload_steps(b):
                for kt in range(KT):
                    def mk(b=b, kt=kt):
                        nc.gpsimd.dma_start(vx[b][:, kt, :], xvr[b][kt, :, :])
                    yield mk

            def vproj_steps(b):
                for tt in range(TT):
                    state = {}

                    def s0(b=b, tt=tt, state=state):
                        state["Vp"] = ps.tile([128, 128], F32, tag="A",
                                              name=f"Vp{b}{tt}")
                        for kt in range(4):
                            nc.tensor.matmul(state["Vp"][:, :],
                                             vx[b][:, kt, tt * 128:(tt + 1) * 128],
                                             wvt[:, kt, :], start=(kt == 0), stop=False)
                    yield s0

                    def s1(b=b, tt=tt, state=state):
                        Vp = state["Vp"]
                        for kt in range(4, KT):
                            nc.tensor.matmul(Vp[:, :],
                                             vx[b][:, kt, tt * 128:(tt + 1) * 128],
                                             wvt[:, kt, :], start=False, stop=False)
                        nc.tensor.matmul(Vp[:, :], onr[0:1, :], bvt[0:1, :],
                                         start=False, stop=True)
                        nc.vector.tensor_copy(vht[b][0][:, tt, 0:64], Vp[:, 0:64])
                        nc.vector.tensor_copy(vht[b][1][:, tt, 0:64], Vp[:, 64:128])
                    yield s1

            def stage2(b, sc, filler=None, pace=1, fin_prev=None, pre=1):
                """One (batch, seq-quarter) attention chunk. AV matmuls run
                `pre` t-tiles behind the logits so ScalarE's exp stream stays
                gapless (deep pre for the first chunk, whose AV inputs wait
                on the xv DMA); fillers are emitted before the AV pair so the
                PE works through them during the exp wait. The normalization
                of the PREVIOUS chunk (fin_prev) is emitted a few tiles into
                this chunk so its engine-queue entries never gate anything."""
                s0, s1 = sc * 512, (sc + 1) * 512
                A0 = ps.tile([128, 512], F32, tag="A0", bufs=1, name=f"A0{b}{sc}")
                A1 = ps.tile([128, 512], F32, tag="A1", bufs=1, name=f"A1{b}{sc}")
                Ps = [None] * TT

                def emit_L(tt):
                    t0, t1 = tt * 128, (tt + 1) * 128
                    L2 = ps.tile([128, 1024], F32, tag="L", name=f"L2{b}{sc}{tt}")
                    nc.tensor.matmul(L2[:, 0:512], khT[b][0:64, t0:t1],
                                     qhT[b][0:64, s0:s1], start=True, stop=True)
                    nc.tensor.matmul(L2[:, 512:1024], khT[b][64:128, t0:t1],
                                     qhT[b][64:128, s0:s1], start=True, stop=True)
                    P = s2p.tile([128, 1024], BF16, tag="P", bufs=8, name=f"P{b}{sc}{tt}")
                    nc.scalar.activation(P[:, :], L2[:, :], F.Exp, scale=0.125)
                    Ps[tt] = P

                def av(tt):
                    st, sp_ = (tt == 0), (tt == TT - 1)
                    nc.tensor.matmul(A0[0:65, :], vht[b][0][:, tt, :],
                                     Ps[tt][:, 0:512], start=st, stop=sp_)
                    nc.tensor.matmul(A1[0:65, :], vht[b][1][:, tt, :],
                                     Ps[tt][:, 512:1024], start=st, stop=sp_)

                for tt in range(pre):
                    emit_L(tt)
                for tt in range(TT):
                    if tt + pre < TT:
                        emit_L(tt + pre)
                    if filler is not None:
                        for _ in range(pace):
                            step = next(filler, None)
                            if step is not None:
                                step()
                    av(tt)
                    if tt == 2 and fin_prev is not None:
                        fin_prev()
                # copy A to SBUF so the psum accumulators free quickly; the
                # rest of the normalization is deferred (returned closure)
                hA0 = s2p.tile([65, 512], F32, tag="hA0", bufs=2, name=f"hA0{b}{sc}")
                hA1 = s2p.tile([65, 512], F32, tag="hA1", bufs=2, name=f"hA1{b}{sc}")
                nc.vector.tensor_copy(hA0[:, :], A0[0:65, :])
                nc.vector.tensor_copy(hA1[:, :], A1[0:65, :])

                def fin():
                    rec = s2p.tile([1, 1024], F32, tag="rec", bufs=1, name=f"rec{b}{sc}")
                    nc.vector.reciprocal(rec[0:1, 0:512], hA0[64:65, :])
                    nc.vector.reciprocal(rec[0:1, 512:1024], hA1[64:65, :])
                    recB = s2p.tile([64, 1024], F32, tag="recB", bufs=1, name=f"recB{b}{sc}")
                    nc.gpsimd.partition_broadcast(recB[:, :], rec[0:1, :])
                    nc.vector.tensor_mul(hN[b][0:64, s0:s1], hA0[0:64, :], recB[:, 0:512])
                    nc.vector.tensor_mul(hN[b][64:128, s0:s1], hA1[0:64, :], recB[:, 512:1024])
                    nc.sync.dma_start(a2a_in[b][4 * b + sc, :, :], hN[b][:, s0:s1])
                return fin

            # ---------- setup + schedule ----------
            qhT = [acp.tile([128, S], BF16, tag=f"qhT{b}", name=f"qhT{b}") for b in range(B)]
            khT = [acp.tile([128, S], BF16, tag=f"khT{b}", name=f"khT{b}") for b in range(B)]
            vht = [[acp.tile([128, TT, 65], BF16, tag=f"vht{b}{h}", name=f"vht{b}{h}")
                    for h in range(2)] for b in range(B)]
            vx = [acp.tile([128, KT, S], BF16, tag="vx", name=f"vx{b}") for b in range(B)]
            hN = [acp.tile([128, S], BF16, tag=f"hN{b}", name=f"hN{b}") for b in range(B)]
            wot = cst.tile([128, KT, D], BF16, tag="wot")
            bqt = cst.tile([128, 1], F32, tag="bqt")
            bkt = cst.tile([128, 1], F32, tag="bkt")
            bvt = cst.tile([1, 128], BF16, tag="bvt")
            bot = cst.tile([1, D], BF16, tag="bot")
            onr = cst.tile([1, 128], BF16, tag="onr")
            zt = cst.tile([128, SQ], BF16, tag="zt")
            hf1 = acp.tile([128, NCORES, SQ], BF16, tag="hf1")
            hf2 = acp.tile([128, NCORES, SQ], BF16, tag="hf2")
            hfs = acp.tile([128, NCORES, SQ], BF16, tag="hfs")
            a2a_in = [dp.tile([NCORES, 128, SQ], BF16, tag=f"a2a_in{b}", name=f"a2a_in{b}")
                      for b in range(B)]
            a2a_out = [dp.tile([NCORES, 128, SQ], BF16, tag=f"a2a_out{b}", name=f"a2a_out{b}")
                       for b in range(B)]

            import itertools
            kgen0 = kproj_full(0)
            next(kgen0)()       # xk0 loads right behind wkt
            nc.sync.dma_start(bkt[:, :], bk_d[:, :])
            nc.sync.dma_start(wqt[:, :, :], wq_d[:, :].rearrange("(kt p) e -> p kt e", p=128))
            nc.sync.dma_start(bqt[:, :], bq_d[:, :])
            qgen00 = qproj_steps(0, 0)
            next(qgen00)()      # xq0 first-half loads
            nc.sync.dma_start(wvt[:, :, :], wv_d[:, :].rearrange("(kt p) e -> p kt e", p=128))
            for t, dtens in ((bvt, bv_d), (bot, bo_d), (onr, onr_d)):
                nc.sync.dma_start(t[:, :], dtens[:, :])
            nc.vector.memset(zt[:, :], 0.0)
            for b in range(B):
                for h in range(2):
                    nc.vector.memset(vht[b][h][:, :, 64:65], 1.0)
            for step in vload_steps(0):
                step()          # xv0 loads behind xq0-sp0
            for step in kgen0:
                step()          # k pieces
            for step in qgen00:
                step()          # q first-half pieces

            # zero-fill the other-batch blocks of a2a_in (must complete
            # before the collectives; emitted late so they don't compete
            # with the startup input loads)
            for b in range(B):
                for r in range(NCORES):
                    if r // 4 != b:
                        nc.gpsimd.dma_start(a2a_in[b][r, :, :], zt[:, :])

            # batch-0 attention; vproj(0) drips into sc0, everything
            # batch-1 (and q0 second half) into sc1-3. All of fillerA must
            # be emitted before stage2(1, 0), else the in-order PE queue
            # deadlocks.
            q01 = qproj_steps(0, 1)
            next(q01)()         # xq0 second-half loads (slots free mid-proj)
            fin00 = stage2(0, 0, itertools.chain(vproj_steps(0)), pace=3, pre=7)
            kg1 = kproj_full(1)
            qg10 = qproj_steps(1, 0)
            # fillers ordered by when their input DMAs can land: q0-sp1 data
            # is resident, vx1/xk1/xq1 stream in behind it
            fillerA = itertools.chain(vload_steps(1),
                                      [next(kg1), next(qg10)],
                                      q01,
                                      vproj_steps(1),
                                      kg1,
                                      qg10)
            fin01 = stage2(0, 1, fillerA, pace=2, fin_prev=fin00)
            fin02 = stage2(0, 2, fillerA, pace=2, fin_prev=fin01)
            fin03 = stage2(0, 3, fillerA, pace=2, fin_prev=fin02)
            for step in fillerA:
                step()
            fillerB = itertools.chain(qproj_steps(1, 1))
            next(fillerB)()     # xq1 second-half loads fire early
            fin10 = stage2(1, 0, fin_prev=fin03)
            # AllToAll-0 emitted after fin03 so its gpsimd-queue entry never
            # sits ahead of fin03's partition_broadcast (deadlock otherwise)
            nc.gpsimd.collective_compute(
                "AllToAll", mybir.AluOpType.bypass,
                replica_groups=[list(range(NCORES))],
                ins=[a2a_in[0].opt()], outs=[a2a_out[0].opt()])
            nc.sync.dma_start(wot[:, :, :],
                              wo_d[:, :].rearrange("(kt p) n -> p kt n", p=128))
            fin11 = stage2(1, 1, fillerB, fin_prev=fin10)
            for step in fillerB:
                step()
            fin12 = stage2(1, 2, fin_prev=fin11)
            fin13 = stage2(1, 3, fin_prev=fin12)
            fin13()
            nc.gpsimd.collective_compute(
                "AllToAll", mybir.AluOpType.bypass,
                replica_groups=[list(range(NCORES))],
                ins=[a2a_in[1].opt()], outs=[a2a_out[1].opt()])

            # ---- tail: small dummy matmuls (N=128, fast to drain) keep the
            # PE warm through the second AllToAll's wait; then both halves'
            # AllToAll outputs are summed (one is zeros) and a single output
            # projection runs at full clock.
            W = ps.tile([128, 128], F32, tag="A0", bufs=1, name="Wwarm")
            for i in range(384):
                nc.tensor.matmul(W[:, :], wot[:, i % KT, 0:128],
                                 wot[:, i % KT, 128:256],
                                 start=(i == 0), stop=(i == 383))
            wsb = s2p.tile([128, 128], BF16, tag="wsb", bufs=1, name="wsb")
            nc.vector.tensor_copy(wsb[:, :], W[:, :])
            nc.sync.dma_start(a2a_in[0][0, :, 0:128], wsb[:, :])

            for p in range(NCORES):
                nc.sync.dma_start(hf1[:, p, :], a2a_out[0][p, :, :])
            for p in range(NCORES):
                nc.sync.dma_start(hf2[:, p, :], a2a_out[1][p, :, :])
            for st in range(4):
                nc.vector.tensor_add(hfs[:, :, st * 128:(st + 1) * 128],
                                     hf1[:, :, st * 128:(st + 1) * 128],
                                     hf2[:, :, st * 128:(st + 1) * 128])
                O = ps.tile([128, 1024], F32, tag="L", name=f"O2_{st}")
                for nn in range(2):
                    n0, n1 = nn * 512, (nn + 1) * 512
                    for kt in range(KT):
                        nc.tensor.matmul(O[:, n0:n1],
                                         hfs[:, kt, st * 128:(st + 1) * 128],
                                         wot[:, kt, n0:n1],
                                         start=(kt == 0), stop=False)
                    nc.tensor.matmul(O[:, n0:n1], onr[0:1, :], bot[0:1, n0:n1],
                                     start=False, stop=True)
                OG = s2p.tile([128, 1024], F32, tag="OG", bufs=2, name=f"OG{st}")
                nc.scalar.activation(OG[:, :], O[:, :], F.Gelu_apprx_sigmoid)
                nc.sync.dma_start(out_d[st * 128:(st + 1) * 128, :], OG[:, :])

    nc.compile()
    return nc


def _in_maps(q, k, v, Wq, bq, Wk, bk, Wv, bv, Wo, bo):
    xq = [np.ascontiguousarray(q[b].T).astype(BF) for b in range(B)]
    xk = [np.ascontiguousarray(k[b].T).astype(BF) for b in range(B)]
    xv = [np.ascontiguousarray(v[b].T).astype(BF) for b in range(B)]
    wo_bf = np.ascontiguousarray(Wo).astype(BF)
    bo_r = np.asarray(bo).reshape(1, D).astype(BF)
    onr = np.ones((1, 128), BF)
    in_maps = []
    for c in range(NCORES):
        hs = slice(2 * c, 2 * c + 2)
        im = {
            "wq": np.ascontiguousarray(Wq[hs].transpose(1, 0, 2).reshape(D, 128)).astype(BF),
            "wk": np.ascontiguousarray(Wk[hs].transpose(1, 0, 2).reshape(D, 128)).astype(BF),
            "wv": np.ascontiguousarray(Wv[hs].transpose(1, 0, 2).reshape(D, 128)).astype(BF),
            "bq": np.asarray(bq[hs]).reshape(128, 1).astype(np.float32),
            "bk": np.asarray(bk[hs]).reshape(128, 1).astype(np.float32),
            "bv": np.asarray(bv[hs]).reshape(1, 128).astype(BF),
            "wo": wo_bf, "bo": bo_r, "onr": onr,
        }
        for b in range(B):
            im[f"xq{b}"] = xq[b]
            im[f"xk{b}"] = xk[b]
            im[f"xv{b}"] = xv[b]
        in_maps.append(im)
    return in_maps


def kernel(q, k, v, mask, Wq, bq, Wk, bk, Wv, bv, Wo, bo):
    if "nc" not in _CACHE:
        _CACHE["nc"] = _build()
    nc = _CACHE["nc"]
    in_maps = _in_maps(q, k, v, Wq, bq, Wk, bk, Wv, bv, Wo, bo)
    res = run_bass_kernel_spmd(nc, in_maps, core_ids=list(range(NCORES)))
    out = np.empty((B, S, D), np.float32)
    for r in range(NCORES):
        bb, jj = r // 4, r % 4
        out[bb, jj * SQ:(jj + 1) * SQ, :] = res.results[r]["out"]
    return out
